# revision 19
# baseline (speedup 1.0000x reference)
"""ColorRandomizer Trainium2 kernel: brightness/contrast/saturation/hue on 8 cores.

Data-parallel: 4 images per core, fp16 storage AND fp16 DRAM I/O (host
converts fp32<->fp16; validated absmax ~8e-3 vs fp32 reference).

Structure (v2, PE-blend + DVE contrast):
  S1  u' = min(bf*cf*x, cf)                DVE TS x3 (immediates baked
      per image; kernel is compiled per factor-set and cached on it)
  S2  mean accum via ACT Identity x3 + GPS all-reduce -> delta [P,1]
  S3  zc = max(u' + delta, 0)              DVE TS [3F] (ptr bias) - the
      old ACT relu is gone, killing the ids->relu->sat convoy on ACT
  S4  zc1 = min(zc, 1) into permuted strip [b|r|g]   DVE TS x3
      blend x3u_p = sum_q M[p,q]*zc1_q     PE: 9 diagonal matmuls/chunk
      into bank-aligned PSUM; ACT Relu exits chunks in place (max0+fp16)
      upper clamp min(.,1) deferred to hue start (DVE TS [3F])
  S5  hue via tent identity, division-free selection (same as v1):
      maxc/minc/cr/s2, masks is_ge [2F], J via copy_predicated,
      i60 = J * exp(-ln(cr+eps)) (single pinned act table), tents on ACT,
      out = clamp(m,1,2)*cr + s2 on DVE.
Software pipeline: blend(i) runs on PE/ACT while hue(i-1/i-2) owns DVE;
head ids of i+2 are queued on ACT before lnexp(i) (DVE consumes i60
~12us later), blend(i+2) right after hue(i).
"""
import sys

for _p in ("/opt/trn_rl_repo",):
    if _p not in sys.path:
        sys.path.append(_p)

import numpy as np
from concourse import bass, bacc, mybir, tile, bass_isa
from concourse.bass_utils import run_bass_kernel_spmd

# Pin every activation we use (Identity/Abs/Ln/Exp) to the one real act
# table that contains them all ("natural_log_exp_and_others"), so the
# table-load pass emits a single load instead of flip-flopping between
# the exp table and the ln table (9 loads x 1.28us on ACT otherwise).
# Python-side view only; the emitted set id still indexes the real
# act_info.json, whose set genuinely holds all these funcs.
import concourse.hw_specs as _hw_specs
import concourse.bacc as _bacc_mod

_ORIG_GET_TABLES = _hw_specs.get_activation_tables


def _pinned_tables(arch):
    tabs = _ORIG_GET_TABLES(arch)
    AF_ = mybir.ActivationFunctionType
    strip = {AF_.Identity, AF_.Relu, AF_.Abs, AF_.Ln, AF_.Exp, AF_.Copy}
    return {
        n: (set(s) if n == "natural_log_exp_and_others" else set(s) - strip)
        for n, s in tabs.items()
    }


_hw_specs.get_activation_tables = _pinned_tables
_bacc_mod.get_activation_tables = _pinned_tables

F32 = mybir.dt.float32
F16 = mybir.dt.float16
I16 = mybir.dt.int16
OP = mybir.AluOpType
AF = mybir.ActivationFunctionType

NIMG = 4          # images per core
H, W = 480, 640
NPIX = H * W      # 307200
F = NPIX // 128   # 2400 free elems per partition per plane
F3 = 3 * F
GRAY_W = (0.299, 0.587, 0.114)
NFAC = 12

# sat/tnt strips use permuted channel order [b|r|g] so that
#   [t2|t3] = [b-r | r-g] and [mr|mg] come from single contiguous [2F] ops.
# POS[c] = strip slot of rgb channel c;  slot p holds channel CHAN[p].
POS = (1, 2, 0)   # r->slot1, g->slot2, b->slot0
CHAN = (2, 0, 1)  # slot0=b, slot1=r, slot2=g
# tent offsets a_c per rgb channel: r:3, g:1, b:-1  (bias col = 6hf + a_c - 3)
HB_COL = (7, 8, 9)  # fac cols for r,g,b bias

TRACE = False     # test.py flips this for profiling
_CACHE = {}


def _build():
    nc = bacc.Bacc(None, target_bir_lowering=False)
    x_h = nc.declare_dram_parameter("x", [NIMG, 3, H, W], F16, isOutput=False)
    fac_h = nc.declare_dram_parameter("fac", [NIMG, NFAC], F32, isOutput=False)
    wd_h = nc.declare_dram_parameter("wd", [NIMG, 9, 128, 128], F16, isOutput=False)
    y_h = nc.declare_dram_parameter("y", [NIMG, 3, H, W], F16, isOutput=True)

    dma = nc.sync  # HWDGE

    # activation float biases must exist as const APs
    for v in (2e-5, -3.0, -1.0):
        t = nc.alloc_sbuf_tensor(f"cst-{v}", [128, 1], F32)
        nc.gpsimd.memset(t.ap(), v)
        nc.const_aps.aps[(F32, v)] = t.ap()
    nc.all_engine_barrier()

    with tile.TileContext(nc) as tc:
        with tc.tile_pool(name="p", bufs=1) as pool, \
             tc.tile_pool(name="ps", bufs=1, space=bass.MemorySpace.PSUM) as psum:
            # broadcast per-image factors to all partitions once (hue biases)
            fac1 = pool.tile([1, NIMG * NFAC], F32)
            dma.dma_start(fac1[:], fac_h[:].flatten()[None, :])
            facb = pool.tile([128, NIMG * NFAC], F32)
            nc.gpsimd.partition_broadcast(facb[:], fac1[:], channels=128)
            # per-image blend matrices as 9 diagonal [128,128] stationaries
            # (coeff * I, host-prepared): out_p = sum_q M[p,q] * zc_q on PE.
            wd = pool.tile([128, NIMG * 9 * 128], F16)
            for ii in range(NIMG):
                for jj in range(9):
                    k0 = (ii * 9 + jj) * 128
                    dma.dma_start(wd[:, k0 : k0 + 128], wd_h[ii, jj])

            def wtile(ii, p, q):
                k0 = (ii * 9 + p * 3 + q) * 128
                return wd[:, k0 : k0 + 128]

            def col(i, k):
                return facb[:, i * NFAC + k : i * NFAC + k + 1]

            def emit_load(i):
                """DMA + S1 u = min(bf*x, 1)."""
                io = pool.tile([128, F3], F16, tag="big", bufs=4)
                cnd = pool.tile([128, F3], F16, tag="cnd", bufs=2)
                sums = pool.tile([128, 4], F32, tag="sums", bufs=3)
                for c in range(3):
                    sl = io[:, c * F : (c + 1) * F]
                    dma.dma_start(
                        sl, x_h[i, c].flatten().rearrange("(p f) -> p f", p=128)
                    )
                    nc.vector.tensor_scalar(sl, sl, col(i, 0), 1.0, OP.mult, OP.min)
                return io, cnd, sums

            def emit_mean(i, io, cnd, sums):
                for c in range(3):
                    nc.scalar.activation(
                        cnd[:, c * F : (c + 1) * F], io[:, c * F : (c + 1) * F],
                        AF.Identity, bias=0.0, scale=1.0,
                        accum_out=sums[:, c : c + 1],
                    )

            def emit_head(i):
                io, cnd, sums = emit_load(i)
                emit_mean(i, io, cnd, sums)
                return io, cnd, sums

            def emit_contrast(i, io, sums):
                """mean-finish + contrast z = cf*u + delta, all DVE/GPS
                (no ACT relu: the max(.,0) folds into the blend strip TS).
                """
                ws = pool.tile([128, 4], F32, tag="ws", bufs=3)
                nc.vector.tensor_scalar(ws[:, 0:1], sums[:, 0:1], GRAY_W[0], None, OP.mult)
                nc.vector.scalar_tensor_tensor(ws[:, 1:2], sums[:, 1:2], GRAY_W[1], ws[:, 0:1], OP.mult, OP.add)
                nc.vector.scalar_tensor_tensor(ws[:, 2:3], sums[:, 2:3], GRAY_W[2], ws[:, 1:2], OP.mult, OP.add)
                ssum = pool.tile([128, 2], F32, tag="ssum", bufs=3)
                nc.gpsimd.partition_all_reduce(ssum[:, 0:1], ws[:, 2:3], 128, bass_isa.ReduceOp.add)
                # delta = ssum * (1-cf)/NPIX
                nc.vector.tensor_tensor(ssum[:, 1:2], ssum[:, 0:1], col(i, 2), OP.mult)
                # z = cf*u + delta in place (dual per-partition scalars)
                nc.vector.tensor_scalar(io[:], io[:], col(i, 1), ssum[:, 1:2], OP.mult, OP.add)

            def emit_blend(i):
                """S4: strip zc1 = min(zc,1) then PE blend: per 480-col
                chunk, 9 diagonal matmuls accumulate out_p = sum_q
                M[p,q]*zc1_q into bank-aligned PSUM (fp32); ACT Relu exits
                each chunk in place (max0 + fp16). Upper clamp deferred to
                hue start so DVE never waits here."""
                io, cnd, sums = heads[i]
                sat = pool.tile([128, F3], F16, tag="sat", bufs=2)
                for c in range(3):
                    p = POS[c]
                    nc.vector.tensor_scalar(
                        sat[:, p * F : (p + 1) * F],
                        io[:, c * F : (c + 1) * F],
                        0.0, 1.0, OP.max, OP.min,
                    )
                CH = 480
                satv = sat[:].rearrange("p (c f) -> p c f", c=3)
                for j in range(F // CH):
                    # rows padded to 512 floats: each out-slot region is
                    # PSUM-bank aligned (2KB) as accumulation requires
                    pt = psum.tile([128, 3, 512], F32, tag="blend", bufs=2)
                    for p in range(3):
                        for q in range(3):
                            nc.tensor.matmul(
                                pt[:, p, 0:CH],
                                wtile(i, p, q),
                                sat[:, q * F + j * CH : q * F + (j + 1) * CH],
                                start=(q == 0), stop=(q == 2),
                            )
                    nc.scalar.activation(
                        satv[:, :, j * CH : (j + 1) * CH], pt[:, :, 0:CH],
                        AF.Relu, bias=0.0, scale=1.0,
                    )
                return sat

            h0 = emit_load(0)
            emit_mean(0, *h0)
            heads = {0: h0, 1: emit_head(1)}
            emit_contrast(0, heads[0][0], heads[0][2])
            sats = {0: emit_blend(0)}
            emit_contrast(1, heads[1][0], heads[1][2])
            sats[1] = emit_blend(1)

            for i in range(NIMG):
                io, cnd, sums = heads[i]
                sat = sats[i]
                nc.vector.tensor_scalar(sat[:], sat[:], 1.0, None, OP.min)
                sb = sat[:, 0:F]       # b
                sr = sat[:, F:2 * F]   # r
                sg = sat[:, 2 * F:3 * F]  # g

                # ---- S5 hue ----
                maxc = pool.tile([128, F], F16, tag="maxc", bufs=1)
                minc = pool.tile([128, F], F16, tag="minc", bufs=1)
                cr = pool.tile([128, F], F16, tag="cr", bufs=1)
                s2 = pool.tile([128, F], F16, tag="s2", bufs=1)
                invc = pool.tile([128, F], F16, tag="invc", bufs=1)

                nc.vector.tensor_tensor(maxc[:], sg, sb, OP.max)
                nc.vector.tensor_tensor(maxc[:], maxc[:], sr, OP.max)
                nc.vector.tensor_tensor(minc[:], sg, sb, OP.min)
                nc.vector.tensor_tensor(minc[:], minc[:], sr, OP.min)
                nc.vector.tensor_tensor(cr[:], maxc[:], minc[:], OP.subtract)
                nc.vector.tensor_tensor(s2[:], minc[:], cr[:], OP.subtract)
                # head of image i+2 now: its ACT mean-ids queue ahead of
                # lnexp (DVE consumes i60 ~12us later), so the serial
                # ids->reduce chain of i+2 starts early.
                if i + 2 < NIMG and (i + 2) not in heads:
                    heads[i + 2] = emit_head(i + 2)
                # invc = 1/(cr+eps) via exp(-ln); fp16 out is safe (<= 5e4)
                lc = pool.tile([128, F], F32, tag="lc", bufs=1)
                nc.scalar.activation(lc[:], cr[:], AF.Ln, bias=2e-5)
                nc.scalar.activation(invc[:], lc[:], AF.Exp, scale=-1.0)

                masks = pool.tile([128, 2 * F], F16, tag="masks", bufs=1)
                mxb = maxc[:][:, None, :].broadcast_to([128, 2, F])
                nc.vector.tensor_tensor(
                    masks[:].bitcast(I16).rearrange("p (c f) -> p c f", c=2),
                    sat[:, F:3 * F].rearrange("p (c f) -> p c f", c=2),
                    mxb, OP.is_ge,
                )
                # candidates: c1 = g-b ; [t2|t3] = [b-r | r-g]
                nc.vector.tensor_tensor(cnd[:, 0:F], sg, sb, OP.subtract)
                nc.vector.tensor_tensor(
                    cnd[:, F:3 * F].rearrange("p (c f) -> p c f", c=2),
                    sat[:, 0:2 * F].rearrange("p (c f) -> p c f", c=2),
                    sat[:, F:3 * F].rearrange("p (c f) -> p c f", c=2),
                    OP.subtract,
                )
                kk = pool.tile([128, 2 * F], F16, tag="kk", bufs=1)
                nc.vector.tensor_scalar(kk[:, 0:F], cr[:], 2.0, None, OP.mult)
                nc.vector.tensor_scalar(kk[:, F:2 * F], kk[:, 0:F], 2.0, None, OP.mult)
                nc.vector.tensor_tensor(cnd[:, F:3 * F], cnd[:, F:3 * F], kk[:], OP.add)
                # J = select into c3 slot: mg -> c2, then mr -> c1
                nc.vector.copy_predicated(cnd[:, 2 * F:3 * F], masks[:, F:2 * F].bitcast(I16), cnd[:, F:2 * F])
                nc.vector.copy_predicated(cnd[:, 2 * F:3 * F], masks[:, 0:F].bitcast(I16), cnd[:, 0:F])
                # i60 = J * invc  (into c1 slot)
                nc.vector.tensor_tensor(cnd[:, 0:F], cnd[:, 2 * F:3 * F], invc[:], OP.mult)

                # tents: B = |i60 + (6hf + a_c - 3)| per slot; m = |B - 3|
                tnt = pool.tile([128, F3], F16, tag="big", bufs=4)
                if i < NIMG - 1:
                    for p in range(3):
                        nc.scalar.activation(
                            tnt[:, p * F : (p + 1) * F], cnd[:, 0:F],
                            AF.Abs, bias=col(i, HB_COL[CHAN[p]]),
                        )
                    nc.scalar.activation(tnt[:], tnt[:], AF.Abs, bias=-3.0)
                    # contrast of image i+2 stays late (after image i's
                    # tents): its DVE smalls fill the tentcl wait below.
                    if i + 2 < NIMG:
                        emit_contrast(i + 2, heads[i + 2][0], heads[i + 2][2])
                    nc.vector.tensor_scalar(tnt[:], tnt[:], 1.0, 2.0, OP.max, OP.min)
                    v3 = lambda t: t[:].rearrange("p (c f) -> p c f", c=3)
                    crb = cr[:][:, None, :].broadcast_to([128, 3, F])
                    s2b = s2[:][:, None, :].broadcast_to([128, 3, F])
                    nc.vector.tensor_tensor(v3(tnt), v3(tnt), crb, OP.mult)
                    nc.vector.tensor_tensor(v3(tnt), v3(tnt), s2b, OP.add)
                    for p in range(3):
                        dma.dma_start(
                            y_h[i, CHAN[p]].flatten().rearrange("(p f) -> p f", p=128),
                            tnt[:, p * F : (p + 1) * F],
                        )
                else:
                    # last image: per-channel tail so ACT(B,m) overlaps DVE
                    for p in range(3):
                        sl = tnt[:, p * F : (p + 1) * F]
                        nc.scalar.activation(sl, cnd[:, 0:F], AF.Abs, bias=col(i, HB_COL[CHAN[p]]))
                        nc.scalar.activation(sl, sl, AF.Abs, bias=-3.0)
                        nc.vector.tensor_scalar(sl, sl, 1.0, 2.0, OP.max, OP.min)
                        nc.vector.tensor_tensor(sl, sl, cr[:], OP.mult)
                        nc.vector.tensor_tensor(sl, sl, s2[:], OP.add)
                        dma.dma_start(
                            y_h[i, CHAN[p]].flatten().rearrange("(p f) -> p f", p=128),
                            sl,
                        )

                # next image's blend runs on PE/ACT while image i+1's hue
                # owns DVE
                if i + 2 < NIMG:
                    sats[i + 2] = emit_blend(i + 2)

    nc.finalize()
    return nc


def _get_nc():
    if "nc" not in _CACHE:
        _CACHE["nc"] = _build()
    return _CACHE["nc"]


def kernel(x, brightness_f, contrast_f, saturation_f, hue_f, num_samples=1, **_):
    x16 = np.ascontiguousarray(np.asarray(x, dtype=np.float16))
    bf = np.asarray(brightness_f, np.float32)
    cf = np.asarray(contrast_f, np.float32)
    sf = np.asarray(saturation_f, np.float32)
    hf = np.asarray(hue_f, np.float32)
    fac = np.stack(
        [
            bf, cf, (1.0 - cf) / np.float32(NPIX), sf,
            np.zeros_like(bf), np.zeros_like(bf), np.zeros_like(bf),
            6.0 * hf + 0.0,   # r: a=3  -> 6hf + a - 3
            6.0 * hf - 2.0,   # g: a=1
            6.0 * hf - 4.0,   # b: a=-1
            np.zeros_like(bf), np.zeros_like(bf),
        ],
        axis=1,
    ).astype(np.float32)

    # blend matrices for the PE: out_p = sum_q M[p,q] * zc1_q over strip
    # slots (slot order [b|r|g]); M[p,q] = sf*d(p==q) + (1-sf)*w_{CHAN[q]}
    wvec = np.array(GRAY_W, np.float32)
    nimg_total = x16.shape[0]
    wdiag = np.zeros((nimg_total, 9, 128, 128), np.float16)
    idx = np.arange(128)
    for i in range(nimg_total):
        for p in range(3):
            for q in range(3):
                coeff = (sf[i] if p == q else 0.0) + (1.0 - sf[i]) * wvec[CHAN[q]]
                wdiag[i, p * 3 + q, idx, idx] = np.float16(coeff)

    nc = _get_nc()
    in_maps = [
        {"x": x16[k * NIMG:(k + 1) * NIMG], "fac": fac[k * NIMG:(k + 1) * NIMG],
         "wd": wdiag[k * NIMG:(k + 1) * NIMG]}
        for k in range(8)
    ]
    res = run_bass_kernel_spmd(nc, in_maps, core_ids=list(range(8)), trace=TRACE)
    if TRACE:
        _CACHE["last"] = res
    out = np.concatenate([res.results[k]["y"] for k in range(8)], axis=0)
    return out.astype(np.float32)


# revision 20
# speedup vs baseline: 1.0686x; 1.0686x over previous
"""ColorRandomizer Trainium2 kernel: brightness/contrast/saturation/hue on 8 cores.

Data-parallel: 4 images per core, fp16 storage AND fp16 DRAM I/O (host
converts fp32<->fp16; validated absmax ~8e-3 vs fp32 reference).

Structure (v2, PE-blend + DVE contrast):
  S1  u' = min(bf*cf*x, cf)                DVE TS x3 (immediates baked
      per image; kernel is compiled per factor-set and cached on it)
  S2  mean accum via ACT Identity x3 + GPS all-reduce -> delta [P,1]
  S3  zc = max(u' + delta, 0)              DVE TS [3F] (ptr bias) - the
      old ACT relu is gone, killing the ids->relu->sat convoy on ACT
  S4  zc1 = min(zc, 1) into permuted strip [b|r|g]   DVE TS x3
      blend x3u_p = sum_q M[p,q]*zc1_q     PE: 9 diagonal matmuls/chunk
      into bank-aligned PSUM; ACT Relu exits chunks in place (max0+fp16)
      upper clamp min(.,1) deferred to hue start (DVE TS [3F])
  S5  hue via tent identity, division-free selection (same as v1):
      maxc/minc/cr/s2, masks is_ge [2F], J via copy_predicated,
      i60 = J * exp(-ln(cr+eps)) (single pinned act table), tents on ACT,
      out = clamp(m,1,2)*cr + s2 on DVE.
Software pipeline: blend(i) runs on PE/ACT while hue(i-1/i-2) owns DVE;
head ids of i+2 are queued on ACT before lnexp(i) (DVE consumes i60
~12us later), blend(i+2) right after hue(i).
"""
import sys

for _p in ("/opt/trn_rl_repo",):
    if _p not in sys.path:
        sys.path.append(_p)

import numpy as np
from concourse import bass, bacc, mybir, tile, bass_isa
from concourse.bass_utils import run_bass_kernel_spmd

# Pin every activation we use (Identity/Abs/Ln/Exp) to the one real act
# table that contains them all ("natural_log_exp_and_others"), so the
# table-load pass emits a single load instead of flip-flopping between
# the exp table and the ln table (9 loads x 1.28us on ACT otherwise).
# Python-side view only; the emitted set id still indexes the real
# act_info.json, whose set genuinely holds all these funcs.
import concourse.hw_specs as _hw_specs
import concourse.bacc as _bacc_mod

_ORIG_GET_TABLES = _hw_specs.get_activation_tables


def _pinned_tables(arch):
    tabs = _ORIG_GET_TABLES(arch)
    AF_ = mybir.ActivationFunctionType
    strip = {AF_.Identity, AF_.Relu, AF_.Abs, AF_.Ln, AF_.Exp, AF_.Copy}
    return {
        n: (set(s) if n == "natural_log_exp_and_others" else set(s) - strip)
        for n, s in tabs.items()
    }


_hw_specs.get_activation_tables = _pinned_tables
_bacc_mod.get_activation_tables = _pinned_tables

F32 = mybir.dt.float32
F16 = mybir.dt.float16
I16 = mybir.dt.int16
OP = mybir.AluOpType
AF = mybir.ActivationFunctionType

NIMG = 4          # images per core
H, W = 480, 640
NPIX = H * W      # 307200
F = NPIX // 128   # 2400 free elems per partition per plane
F3 = 3 * F
GRAY_W = (0.299, 0.587, 0.114)
NFAC = 12

# sat/tnt strips use permuted channel order [b|r|g] so that
#   [t2|t3] = [b-r | r-g] and [mr|mg] come from single contiguous [2F] ops.
# POS[c] = strip slot of rgb channel c;  slot p holds channel CHAN[p].
POS = (1, 2, 0)   # r->slot1, g->slot2, b->slot0
CHAN = (2, 0, 1)  # slot0=b, slot1=r, slot2=g
# tent offsets a_c per rgb channel: r:3, g:1, b:-1  (bias col = 6hf + a_c - 3)
HB_COL = (7, 8, 9)  # fac cols for r,g,b bias

TRACE = False     # test.py flips this for profiling
_CACHE = {}


def _build():
    nc = bacc.Bacc(None, target_bir_lowering=False)
    x_h = nc.declare_dram_parameter("x", [NIMG, 3, H, W], F16, isOutput=False)
    fac_h = nc.declare_dram_parameter("fac", [NIMG, NFAC], F32, isOutput=False)
    wd_h = nc.declare_dram_parameter("wd", [NIMG, 9, 128, 128], F16, isOutput=False)
    y_h = nc.declare_dram_parameter("y", [NIMG, 3, H, W], F16, isOutput=True)

    dma = nc.sync  # HWDGE

    # activation float biases must exist as const APs
    for v in (2e-5, -3.0, -1.0):
        t = nc.alloc_sbuf_tensor(f"cst-{v}", [128, 1], F32)
        nc.gpsimd.memset(t.ap(), v)
        nc.const_aps.aps[(F32, v)] = t.ap()
    nc.all_engine_barrier()

    with tile.TileContext(nc) as tc:
        with tc.tile_pool(name="p", bufs=1) as pool, \
             tc.tile_pool(name="ps", bufs=1, space=bass.MemorySpace.PSUM) as psum:
            # broadcast per-image factors to all partitions once (hue biases)
            fac1 = pool.tile([1, NIMG * NFAC], F32)
            dma.dma_start(fac1[:], fac_h[:].flatten()[None, :])
            facb = pool.tile([128, NIMG * NFAC], F32)
            nc.gpsimd.partition_broadcast(facb[:], fac1[:], channels=128)
            # per-image blend matrices as 9 diagonal [128,128] stationaries
            # (coeff * I, host-prepared): out_p = sum_q M[p,q] * zc_q on PE.
            wd = pool.tile([128, NIMG * 9 * 128], F16)

            def emit_wd_dmas():
                for ii in range(NIMG):
                    for jj in range(9):
                        k0 = (ii * 9 + jj) * 128
                        dma.dma_start(wd[:, k0 : k0 + 128], wd_h[ii, jj])

            def wtile(ii, p, q):
                k0 = (ii * 9 + p * 3 + q) * 128
                return wd[:, k0 : k0 + 128]

            def col(i, k):
                return facb[:, i * NFAC + k : i * NFAC + k + 1]

            def emit_load(i):
                """DMA + S1 u = min(bf*x, 1)."""
                io = pool.tile([128, F3], F16, tag="big", bufs=4)
                cnd = pool.tile([128, F3], F16, tag="cnd", bufs=2)
                sums = pool.tile([128, 4], F32, tag="sums", bufs=3)
                for c in range(3):
                    sl = io[:, c * F : (c + 1) * F]
                    dma.dma_start(
                        sl, x_h[i, c].flatten().rearrange("(p f) -> p f", p=128)
                    )
                    nc.vector.tensor_scalar(sl, sl, col(i, 0), 1.0, OP.mult, OP.min)
                return io, cnd, sums

            def emit_mean(i, io, cnd, sums):
                for c in range(3):
                    nc.scalar.activation(
                        cnd[:, c * F : (c + 1) * F], io[:, c * F : (c + 1) * F],
                        AF.Identity, bias=0.0, scale=1.0,
                        accum_out=sums[:, c : c + 1],
                    )

            def emit_head(i):
                io, cnd, sums = emit_load(i)
                emit_mean(i, io, cnd, sums)
                return io, cnd, sums

            def emit_contrast(i, io, sums):
                """mean-finish + contrast z = cf*u + delta, all DVE/GPS
                (no ACT relu: the max(.,0) folds into the blend strip TS).
                """
                ws = pool.tile([128, 4], F32, tag="ws", bufs=3)
                nc.vector.tensor_scalar(ws[:, 0:1], sums[:, 0:1], GRAY_W[0], None, OP.mult)
                nc.vector.scalar_tensor_tensor(ws[:, 1:2], sums[:, 1:2], GRAY_W[1], ws[:, 0:1], OP.mult, OP.add)
                nc.vector.scalar_tensor_tensor(ws[:, 2:3], sums[:, 2:3], GRAY_W[2], ws[:, 1:2], OP.mult, OP.add)
                ssum = pool.tile([128, 2], F32, tag="ssum", bufs=3)
                nc.gpsimd.partition_all_reduce(ssum[:, 0:1], ws[:, 2:3], 128, bass_isa.ReduceOp.add)
                # delta = ssum * (1-cf)/NPIX
                nc.vector.tensor_tensor(ssum[:, 1:2], ssum[:, 0:1], col(i, 2), OP.mult)
                # z = cf*u + delta in place (dual per-partition scalars)
                nc.vector.tensor_scalar(io[:], io[:], col(i, 1), ssum[:, 1:2], OP.mult, OP.add)

            def emit_blend(i):
                """S4: strip zc1 = min(zc,1) then PE blend: per 480-col
                chunk, 9 diagonal matmuls accumulate out_p = sum_q
                M[p,q]*zc1_q into bank-aligned PSUM (fp32); ACT Relu exits
                each chunk in place (max0 + fp16). Upper clamp deferred to
                hue start so DVE never waits here."""
                io, cnd, sums = heads[i]
                sat = pool.tile([128, F3], F16, tag="sat", bufs=2)
                for c in range(3):
                    p = POS[c]
                    nc.vector.tensor_scalar(
                        sat[:, p * F : (p + 1) * F],
                        io[:, c * F : (c + 1) * F],
                        0.0, 1.0, OP.max, OP.min,
                    )
                CH = 480
                satv = sat[:].rearrange("p (c f) -> p c f", c=3)
                for j in range(F // CH):
                    # rows padded to 512 floats: each out-slot region is
                    # PSUM-bank aligned (2KB) as accumulation requires
                    pt = psum.tile([128, 3, 512], F32, tag="blend", bufs=2)
                    for p in range(3):
                        for q in range(3):
                            nc.tensor.matmul(
                                pt[:, p, 0:CH],
                                wtile(i, p, q),
                                sat[:, q * F + j * CH : q * F + (j + 1) * CH],
                                start=(q == 0), stop=(q == 2),
                            )
                    nc.scalar.activation(
                        satv[:, :, j * CH : (j + 1) * CH], pt[:, :, 0:CH],
                        AF.Relu, bias=0.0, scale=1.0,
                    )
                return sat

            h0 = emit_load(0)
            emit_mean(0, *h0)
            h1 = emit_load(1)
            heads = {0: h0, 1: h1}
            emit_wd_dmas()
            emit_contrast(0, heads[0][0], heads[0][2])
            sats = {0: emit_blend(0)}
            emit_mean(1, *h1)
            emit_contrast(1, heads[1][0], heads[1][2])
            sats[1] = emit_blend(1)

            for i in range(NIMG):
                io, cnd, sums = heads[i]
                sat = sats[i]
                nc.vector.tensor_scalar(sat[:], sat[:], 1.0, None, OP.min)
                sb = sat[:, 0:F]       # b
                sr = sat[:, F:2 * F]   # r
                sg = sat[:, 2 * F:3 * F]  # g

                # ---- S5 hue ----
                maxc = pool.tile([128, F], F16, tag="maxc", bufs=1)
                minc = pool.tile([128, F], F16, tag="minc", bufs=1)
                cr = pool.tile([128, F], F16, tag="cr", bufs=1)
                s2 = pool.tile([128, F], F16, tag="s2", bufs=1)
                invc = pool.tile([128, F], F16, tag="invc", bufs=1)

                nc.vector.tensor_tensor(maxc[:], sg, sb, OP.max)
                nc.vector.tensor_tensor(maxc[:], maxc[:], sr, OP.max)
                nc.vector.tensor_tensor(minc[:], sg, sb, OP.min)
                nc.vector.tensor_tensor(minc[:], minc[:], sr, OP.min)
                nc.vector.tensor_tensor(cr[:], maxc[:], minc[:], OP.subtract)
                nc.vector.tensor_tensor(s2[:], minc[:], cr[:], OP.subtract)
                # head of image i+2 now: its ACT mean-ids queue ahead of
                # lnexp (DVE consumes i60 ~12us later), so the serial
                # ids->reduce chain of i+2 starts early.
                if i + 2 < NIMG and (i + 2) not in heads:
                    heads[i + 2] = emit_head(i + 2)
                # invc = 1/(cr+eps) via exp(-ln); fp16 out is safe (<= 5e4)
                lc = pool.tile([128, F], F32, tag="lc", bufs=1)
                nc.scalar.activation(lc[:], cr[:], AF.Ln, bias=2e-5)
                nc.scalar.activation(invc[:], lc[:], AF.Exp, scale=-1.0)

                masks = pool.tile([128, 2 * F], F16, tag="masks", bufs=1)
                mxb = maxc[:][:, None, :].broadcast_to([128, 2, F])
                nc.vector.tensor_tensor(
                    masks[:].bitcast(I16).rearrange("p (c f) -> p c f", c=2),
                    sat[:, F:3 * F].rearrange("p (c f) -> p c f", c=2),
                    mxb, OP.is_ge,
                )
                # candidates: c1 = g-b ; [t2|t3] = [b-r | r-g]
                nc.vector.tensor_tensor(cnd[:, 0:F], sg, sb, OP.subtract)
                nc.vector.tensor_tensor(
                    cnd[:, F:3 * F].rearrange("p (c f) -> p c f", c=2),
                    sat[:, 0:2 * F].rearrange("p (c f) -> p c f", c=2),
                    sat[:, F:3 * F].rearrange("p (c f) -> p c f", c=2),
                    OP.subtract,
                )
                kk = pool.tile([128, 2 * F], F16, tag="kk", bufs=1)
                nc.vector.tensor_scalar(kk[:, 0:F], cr[:], 2.0, None, OP.mult)
                nc.vector.tensor_scalar(kk[:, F:2 * F], kk[:, 0:F], 2.0, None, OP.mult)
                nc.vector.tensor_tensor(cnd[:, F:3 * F], cnd[:, F:3 * F], kk[:], OP.add)
                # J = select into c3 slot: mg -> c2, then mr -> c1
                nc.vector.copy_predicated(cnd[:, 2 * F:3 * F], masks[:, F:2 * F].bitcast(I16), cnd[:, F:2 * F])
                nc.vector.copy_predicated(cnd[:, 2 * F:3 * F], masks[:, 0:F].bitcast(I16), cnd[:, 0:F])
                # i60 = J * invc  (into c1 slot)
                nc.vector.tensor_tensor(cnd[:, 0:F], cnd[:, 2 * F:3 * F], invc[:], OP.mult)

                # tents: B = |i60 + (6hf + a_c - 3)| per slot; m = |B - 3|
                tnt = pool.tile([128, F3], F16, tag="big", bufs=4)
                if i < NIMG - 1:
                    for p in range(3):
                        nc.scalar.activation(
                            tnt[:, p * F : (p + 1) * F], cnd[:, 0:F],
                            AF.Abs, bias=col(i, HB_COL[CHAN[p]]),
                        )
                    nc.scalar.activation(tnt[:], tnt[:], AF.Abs, bias=-3.0)
                    # contrast of image i+2 stays late (after image i's
                    # tents): its DVE smalls fill the tentcl wait below.
                    if i + 2 < NIMG:
                        emit_contrast(i + 2, heads[i + 2][0], heads[i + 2][2])
                    nc.vector.tensor_scalar(tnt[:], tnt[:], 1.0, 2.0, OP.max, OP.min)
                    v3 = lambda t: t[:].rearrange("p (c f) -> p c f", c=3)
                    crb = cr[:][:, None, :].broadcast_to([128, 3, F])
                    s2b = s2[:][:, None, :].broadcast_to([128, 3, F])
                    nc.vector.tensor_tensor(v3(tnt), v3(tnt), crb, OP.mult)
                    nc.vector.tensor_tensor(v3(tnt), v3(tnt), s2b, OP.add)
                    for p in range(3):
                        dma.dma_start(
                            y_h[i, CHAN[p]].flatten().rearrange("(p f) -> p f", p=128),
                            tnt[:, p * F : (p + 1) * F],
                        )
                else:
                    # last image: per-channel tail so ACT(B,m) overlaps DVE
                    for p in range(3):
                        sl = tnt[:, p * F : (p + 1) * F]
                        nc.scalar.activation(sl, cnd[:, 0:F], AF.Abs, bias=col(i, HB_COL[CHAN[p]]))
                        nc.scalar.activation(sl, sl, AF.Abs, bias=-3.0)
                        nc.vector.tensor_scalar(sl, sl, 1.0, 2.0, OP.max, OP.min)
                        nc.vector.tensor_tensor(sl, sl, cr[:], OP.mult)
                        nc.vector.tensor_tensor(sl, sl, s2[:], OP.add)
                        dma.dma_start(
                            y_h[i, CHAN[p]].flatten().rearrange("(p f) -> p f", p=128),
                            sl,
                        )

                # next image's blend runs on PE/ACT while image i+1's hue
                # owns DVE
                if i + 2 < NIMG:
                    sats[i + 2] = emit_blend(i + 2)

    nc.finalize()
    return nc


def _get_nc():
    if "nc" not in _CACHE:
        _CACHE["nc"] = _build()
    return _CACHE["nc"]


def kernel(x, brightness_f, contrast_f, saturation_f, hue_f, num_samples=1, **_):
    x16 = np.ascontiguousarray(np.asarray(x, dtype=np.float16))
    bf = np.asarray(brightness_f, np.float32)
    cf = np.asarray(contrast_f, np.float32)
    sf = np.asarray(saturation_f, np.float32)
    hf = np.asarray(hue_f, np.float32)
    fac = np.stack(
        [
            bf, cf, (1.0 - cf) / np.float32(NPIX), sf,
            np.zeros_like(bf), np.zeros_like(bf), np.zeros_like(bf),
            6.0 * hf + 0.0,   # r: a=3  -> 6hf + a - 3
            6.0 * hf - 2.0,   # g: a=1
            6.0 * hf - 4.0,   # b: a=-1
            np.zeros_like(bf), np.zeros_like(bf),
        ],
        axis=1,
    ).astype(np.float32)

    # blend matrices for the PE: out_p = sum_q M[p,q] * zc1_q over strip
    # slots (slot order [b|r|g]); M[p,q] = sf*d(p==q) + (1-sf)*w_{CHAN[q]}
    wvec = np.array(GRAY_W, np.float32)
    nimg_total = x16.shape[0]
    wdiag = np.zeros((nimg_total, 9, 128, 128), np.float16)
    idx = np.arange(128)
    for i in range(nimg_total):
        for p in range(3):
            for q in range(3):
                coeff = (sf[i] if p == q else 0.0) + (1.0 - sf[i]) * wvec[CHAN[q]]
                wdiag[i, p * 3 + q, idx, idx] = np.float16(coeff)

    nc = _get_nc()
    in_maps = [
        {"x": x16[k * NIMG:(k + 1) * NIMG], "fac": fac[k * NIMG:(k + 1) * NIMG],
         "wd": wdiag[k * NIMG:(k + 1) * NIMG]}
        for k in range(8)
    ]
    res = run_bass_kernel_spmd(nc, in_maps, core_ids=list(range(8)), trace=TRACE)
    if TRACE:
        _CACHE["last"] = res
    out = np.concatenate([res.results[k]["y"] for k in range(8)], axis=0)
    return out.astype(np.float32)


# revision 21
# speedup vs baseline: 1.1647x; 1.0899x over previous
"""ColorRandomizer Trainium2 kernel: brightness/contrast/saturation/hue on 8 cores.

Data-parallel: 4 images per core, fp16 storage AND fp16 DRAM I/O (host
converts fp32<->fp16; validated absmax ~8e-3 vs fp32 reference).

Structure (v2, PE-blend + DVE contrast):
  S1  u' = min(bf*cf*x, cf)                DVE TS x3 (immediates baked
      per image; kernel is compiled per factor-set and cached on it)
  S2  mean accum via ACT Identity x3 + GPS all-reduce -> delta [P,1]
  S3  zc = max(u' + delta, 0)              DVE TS [3F] (ptr bias) - the
      old ACT relu is gone, killing the ids->relu->sat convoy on ACT
  S4  zc1 = min(zc, 1) into permuted strip [b|r|g]   DVE TS x3
      blend x3u_p = sum_q M[p,q]*zc1_q     PE: 9 diagonal matmuls/chunk
      into bank-aligned PSUM; ACT Relu exits chunks in place (max0+fp16)
      upper clamp min(.,1) deferred to hue start (DVE TS [3F])
  S5  hue via tent identity, division-free selection (same as v1):
      maxc/minc/cr/s2, masks is_ge [2F], J via copy_predicated,
      i60 = J * exp(-ln(cr+eps)) (single pinned act table), tents on ACT,
      out = clamp(m,1,2)*cr + s2 on DVE.
Software pipeline: blend(i) runs on PE/ACT while hue(i-1/i-2) owns DVE;
head ids of i+2 are queued on ACT before lnexp(i) (DVE consumes i60
~12us later), blend(i+2) right after hue(i).
"""
import sys

for _p in ("/opt/trn_rl_repo",):
    if _p not in sys.path:
        sys.path.append(_p)

import numpy as np
from concourse import bass, bacc, mybir, tile, bass_isa
from concourse.bass_utils import run_bass_kernel_spmd

# Pin every activation we use (Identity/Abs/Ln/Exp) to the one real act
# table that contains them all ("natural_log_exp_and_others"), so the
# table-load pass emits a single load instead of flip-flopping between
# the exp table and the ln table (9 loads x 1.28us on ACT otherwise).
# Python-side view only; the emitted set id still indexes the real
# act_info.json, whose set genuinely holds all these funcs.
import concourse.hw_specs as _hw_specs
import concourse.bacc as _bacc_mod

_ORIG_GET_TABLES = _hw_specs.get_activation_tables


def _pinned_tables(arch):
    tabs = _ORIG_GET_TABLES(arch)
    AF_ = mybir.ActivationFunctionType
    strip = {AF_.Identity, AF_.Relu, AF_.Abs, AF_.Ln, AF_.Exp, AF_.Copy}
    return {
        n: (set(s) if n == "natural_log_exp_and_others" else set(s) - strip)
        for n, s in tabs.items()
    }


_hw_specs.get_activation_tables = _pinned_tables
_bacc_mod.get_activation_tables = _pinned_tables

F32 = mybir.dt.float32
F16 = mybir.dt.float16
I16 = mybir.dt.int16
OP = mybir.AluOpType
AF = mybir.ActivationFunctionType

NIMG = 4          # images per core
H, W = 480, 640
NPIX = H * W      # 307200
F = NPIX // 128   # 2400 free elems per partition per plane
F3 = 3 * F
GRAY_W = (0.299, 0.587, 0.114)
NFAC = 12

# sat/tnt strips use permuted channel order [b|r|g] so that
#   [t2|t3] = [b-r | r-g] and [mr|mg] come from single contiguous [2F] ops.
# POS[c] = strip slot of rgb channel c;  slot p holds channel CHAN[p].
POS = (1, 2, 0)   # r->slot1, g->slot2, b->slot0
CHAN = (2, 0, 1)  # slot0=b, slot1=r, slot2=g
# tent offsets a_c per rgb channel: r:3, g:1, b:-1  (bias col = 6hf + a_c - 3)
HB_COL = (7, 8, 9)  # fac cols for r,g,b bias

TRACE = False     # test.py flips this for profiling
_CACHE = {}


def _build():
    nc = bacc.Bacc(None, target_bir_lowering=False)
    x_h = nc.declare_dram_parameter("x", [NIMG, 3, H, W], F16, isOutput=False)
    fac_h = nc.declare_dram_parameter("fac", [NIMG, NFAC], F32, isOutput=False)
    wd_h = nc.declare_dram_parameter("wd", [NIMG, 9, 128, 128], F16, isOutput=False)
    y_h = nc.declare_dram_parameter("y", [NIMG, 3, H, W], F16, isOutput=True)

    dma = nc.sync  # HWDGE

    # activation float biases must exist as const APs
    for v in (2e-5, -3.0, -1.0):
        t = nc.alloc_sbuf_tensor(f"cst-{v}", [128, 1], F32)
        nc.gpsimd.memset(t.ap(), v)
        nc.const_aps.aps[(F32, v)] = t.ap()
    nc.all_engine_barrier()

    with tile.TileContext(nc) as tc:
        with tc.tile_pool(name="p", bufs=1) as pool, \
             tc.tile_pool(name="ps", bufs=1, space=bass.MemorySpace.PSUM) as psum:
            # broadcast per-image factors to all partitions once (hue biases)
            fac1 = pool.tile([1, NIMG * NFAC], F32)
            dma.dma_start(fac1[:], fac_h[:].flatten()[None, :])
            facb = pool.tile([128, NIMG * NFAC], F32)
            nc.gpsimd.partition_broadcast(facb[:], fac1[:], channels=128)
            # per-image blend matrices as 9 diagonal [128,128] stationaries
            # (coeff * I, host-prepared): out_p = sum_q M[p,q] * zc_q on PE.
            wd = pool.tile([128, NIMG * 9 * 128], F16)

            def emit_wd_dmas():
                for ii in range(NIMG):
                    for jj in range(9):
                        k0 = (ii * 9 + jj) * 128
                        dma.dma_start(wd[:, k0 : k0 + 128], wd_h[ii, jj])

            def wtile(ii, p, q):
                k0 = (ii * 9 + p * 3 + q) * 128
                return wd[:, k0 : k0 + 128]

            def col(i, k):
                return facb[:, i * NFAC + k : i * NFAC + k + 1]

            def emit_load(i):
                """DMA + S1 u = min(bf*x, 1)."""
                io = pool.tile([128, F3], F16, tag="big", bufs=4)
                cnd = pool.tile([128, F3], F16, tag="cnd", bufs=2)
                sums = pool.tile([128, 4], F32, tag="sums", bufs=3)
                for c in range(3):
                    sl = io[:, c * F : (c + 1) * F]
                    dma.dma_start(
                        sl, x_h[i, c].flatten().rearrange("(p f) -> p f", p=128)
                    )
                    nc.vector.tensor_scalar(sl, sl, col(i, 0), 1.0, OP.mult, OP.min)
                return io, cnd, sums

            def emit_mean(i, io, cnd, sums):
                for c in range(3):
                    nc.scalar.activation(
                        cnd[:, c * F : (c + 1) * F], io[:, c * F : (c + 1) * F],
                        AF.Identity, bias=0.0, scale=1.0,
                        accum_out=sums[:, c : c + 1],
                    )

            def emit_head(i):
                io, cnd, sums = emit_load(i)
                emit_mean(i, io, cnd, sums)
                return io, cnd, sums

            def emit_contrast(i, io, sums):
                """mean-finish + contrast z = cf*u + delta, all DVE/GPS
                (no ACT relu: the max(.,0) folds into the blend strip TS).
                """
                ws = pool.tile([128, 4], F32, tag="ws", bufs=3)
                nc.vector.tensor_scalar(ws[:, 0:1], sums[:, 0:1], GRAY_W[0], None, OP.mult)
                nc.vector.scalar_tensor_tensor(ws[:, 1:2], sums[:, 1:2], GRAY_W[1], ws[:, 0:1], OP.mult, OP.add)
                nc.vector.scalar_tensor_tensor(ws[:, 2:3], sums[:, 2:3], GRAY_W[2], ws[:, 1:2], OP.mult, OP.add)
                ssum = pool.tile([128, 2], F32, tag="ssum", bufs=3)
                nc.gpsimd.partition_all_reduce(ssum[:, 0:1], ws[:, 2:3], 128, bass_isa.ReduceOp.add)
                # delta = ssum * (1-cf)/NPIX
                nc.vector.tensor_tensor(ssum[:, 1:2], ssum[:, 0:1], col(i, 2), OP.mult)
                # z = cf*u + delta in place (dual per-partition scalars)
                nc.vector.tensor_scalar(io[:], io[:], col(i, 1), ssum[:, 1:2], OP.mult, OP.add)

            def emit_blend(i):
                """S4: strip zc1 = min(zc,1) then PE blend: per 480-col
                chunk, 9 diagonal matmuls accumulate out_p = sum_q
                M[p,q]*zc1_q into bank-aligned PSUM (fp32); ACT Relu exits
                each chunk in place (max0 + fp16). Upper clamp deferred to
                hue start so DVE never waits here."""
                io, cnd, sums = heads[i]
                sat = pool.tile([128, F3], F16, tag="sat", bufs=2)
                for c in range(3):
                    p = POS[c]
                    nc.vector.tensor_scalar(
                        sat[:, p * F : (p + 1) * F],
                        io[:, c * F : (c + 1) * F],
                        0.0, 1.0, OP.max, OP.min,
                    )
                CH = 480
                satv = sat[:].rearrange("p (c f) -> p c f", c=3)
                for j in range(F // CH):
                    # rows padded to 512 floats: each out-slot region is
                    # PSUM-bank aligned (2KB) as accumulation requires
                    pt = psum.tile([128, 3, 512], F32, tag="blend", bufs=2)
                    for p in range(3):
                        for q in range(3):
                            nc.tensor.matmul(
                                pt[:, p, 0:CH],
                                wtile(i, p, q),
                                sat[:, q * F + j * CH : q * F + (j + 1) * CH],
                                start=(q == 0), stop=(q == 2),
                            )
                    nc.scalar.activation(
                        satv[:, :, j * CH : (j + 1) * CH], pt[:, :, 0:CH],
                        AF.Relu, bias=0.0, scale=1.0,
                    )
                return sat

            h0 = emit_load(0)
            emit_mean(0, *h0)
            h1 = emit_load(1)
            heads = {0: h0, 1: h1}
            emit_wd_dmas()
            emit_contrast(0, heads[0][0], heads[0][2])
            sats = {0: emit_blend(0)}
            emit_mean(1, *h1)
            emit_contrast(1, heads[1][0], heads[1][2])
            sats[1] = emit_blend(1)

            for i in range(NIMG):
                io, cnd, sums = heads[i]
                sat = sats[i]
                nc.vector.tensor_scalar(sat[:], sat[:], 1.0, None, OP.min)
                sb = sat[:, 0:F]       # b
                sr = sat[:, F:2 * F]   # r
                sg = sat[:, 2 * F:3 * F]  # g

                # ---- S5 hue ----
                maxc = pool.tile([128, F], F16, tag="maxc", bufs=1)
                minc = pool.tile([128, F], F16, tag="minc", bufs=1)
                cr = pool.tile([128, F], F16, tag="cr", bufs=1)
                s2 = pool.tile([128, F], F16, tag="s2", bufs=1)
                invc = pool.tile([128, F], F16, tag="invc", bufs=1)

                nc.vector.tensor_tensor(maxc[:], sg, sb, OP.max)
                nc.vector.tensor_tensor(maxc[:], maxc[:], sr, OP.max)
                nc.vector.tensor_tensor(minc[:], sg, sb, OP.min)
                nc.vector.tensor_tensor(minc[:], minc[:], sr, OP.min)
                nc.vector.tensor_tensor(cr[:], maxc[:], minc[:], OP.subtract)
                nc.vector.tensor_tensor(s2[:], minc[:], cr[:], OP.subtract)
                # invc = 1/(cr+eps) via exp(-ln); fp16 out is safe (<= 5e4)
                lc = pool.tile([128, F], F32, tag="lc", bufs=1)
                nc.scalar.activation(lc[:], cr[:], AF.Ln, bias=2e-5)
                nc.scalar.activation(invc[:], lc[:], AF.Exp, scale=-1.0)

                masks = pool.tile([128, 2 * F], F16, tag="masks", bufs=1)
                mxb = maxc[:][:, None, :].broadcast_to([128, 2, F])
                nc.vector.tensor_tensor(
                    masks[:].bitcast(I16).rearrange("p (c f) -> p c f", c=2),
                    sat[:, F:3 * F].rearrange("p (c f) -> p c f", c=2),
                    mxb, OP.is_ge,
                )
                # candidates: c1 = g-b ; [t2|t3] = [b-r | r-g]
                nc.vector.tensor_tensor(cnd[:, 0:F], sg, sb, OP.subtract)
                nc.vector.tensor_tensor(
                    cnd[:, F:3 * F].rearrange("p (c f) -> p c f", c=2),
                    sat[:, 0:2 * F].rearrange("p (c f) -> p c f", c=2),
                    sat[:, F:3 * F].rearrange("p (c f) -> p c f", c=2),
                    OP.subtract,
                )
                kk = pool.tile([128, 2 * F], F16, tag="kk", bufs=1)
                nc.vector.tensor_scalar(kk[:, 0:F], cr[:], 2.0, None, OP.mult)
                nc.vector.tensor_scalar(kk[:, F:2 * F], kk[:, 0:F], 2.0, None, OP.mult)
                nc.vector.tensor_tensor(cnd[:, F:3 * F], cnd[:, F:3 * F], kk[:], OP.add)
                # J = select into c3 slot: mg -> c2, then mr -> c1
                nc.vector.copy_predicated(cnd[:, 2 * F:3 * F], masks[:, F:2 * F].bitcast(I16), cnd[:, F:2 * F])
                nc.vector.copy_predicated(cnd[:, 2 * F:3 * F], masks[:, 0:F].bitcast(I16), cnd[:, 0:F])
                # i60 = J * invc  (into c1 slot)
                nc.vector.tensor_tensor(cnd[:, 0:F], cnd[:, 2 * F:3 * F], invc[:], OP.mult)

                # tents: B = |i60 + (6hf + a_c - 3)| per slot; m = |B - 3|
                tnt = pool.tile([128, F3], F16, tag="big", bufs=4)
                if i < NIMG - 1:
                    for p in range(3):
                        nc.scalar.activation(
                            tnt[:, p * F : (p + 1) * F], cnd[:, 0:F],
                            AF.Abs, bias=col(i, HB_COL[CHAN[p]]),
                        )
                    nc.scalar.activation(tnt[:], tnt[:], AF.Abs, bias=-3.0)
                    # head + contrast of image i+2 go here, after image i's
                    # tents are queued: ACT's chain for image i is then just
                    # lnexp+tents (~18us), inside DVE's mask/cand window, and
                    # the i+2 DVE smalls fill the tentcl wait below.
                    if i + 2 < NIMG:
                        heads[i + 2] = emit_head(i + 2)
                        emit_contrast(i + 2, heads[i + 2][0], heads[i + 2][2])
                    nc.vector.tensor_scalar(tnt[:], tnt[:], 1.0, 2.0, OP.max, OP.min)
                    v3 = lambda t: t[:].rearrange("p (c f) -> p c f", c=3)
                    crb = cr[:][:, None, :].broadcast_to([128, 3, F])
                    s2b = s2[:][:, None, :].broadcast_to([128, 3, F])
                    nc.vector.tensor_tensor(v3(tnt), v3(tnt), crb, OP.mult)
                    nc.vector.tensor_tensor(v3(tnt), v3(tnt), s2b, OP.add)
                    for p in range(3):
                        dma.dma_start(
                            y_h[i, CHAN[p]].flatten().rearrange("(p f) -> p f", p=128),
                            tnt[:, p * F : (p + 1) * F],
                        )
                else:
                    # last image: per-channel tail so ACT(B,m) overlaps DVE
                    for p in range(3):
                        sl = tnt[:, p * F : (p + 1) * F]
                        nc.scalar.activation(sl, cnd[:, 0:F], AF.Abs, bias=col(i, HB_COL[CHAN[p]]))
                        nc.scalar.activation(sl, sl, AF.Abs, bias=-3.0)
                        nc.vector.tensor_scalar(sl, sl, 1.0, 2.0, OP.max, OP.min)
                        nc.vector.tensor_tensor(sl, sl, cr[:], OP.mult)
                        nc.vector.tensor_tensor(sl, sl, s2[:], OP.add)
                        dma.dma_start(
                            y_h[i, CHAN[p]].flatten().rearrange("(p f) -> p f", p=128),
                            sl,
                        )

                # next image's blend runs on PE/ACT while image i+1's hue
                # owns DVE
                if i + 2 < NIMG:
                    sats[i + 2] = emit_blend(i + 2)

    nc.finalize()
    return nc


def _get_nc():
    if "nc" not in _CACHE:
        _CACHE["nc"] = _build()
    return _CACHE["nc"]


def kernel(x, brightness_f, contrast_f, saturation_f, hue_f, num_samples=1, **_):
    x16 = np.ascontiguousarray(np.asarray(x, dtype=np.float16))
    bf = np.asarray(brightness_f, np.float32)
    cf = np.asarray(contrast_f, np.float32)
    sf = np.asarray(saturation_f, np.float32)
    hf = np.asarray(hue_f, np.float32)
    fac = np.stack(
        [
            bf, cf, (1.0 - cf) / np.float32(NPIX), sf,
            np.zeros_like(bf), np.zeros_like(bf), np.zeros_like(bf),
            6.0 * hf + 0.0,   # r: a=3  -> 6hf + a - 3
            6.0 * hf - 2.0,   # g: a=1
            6.0 * hf - 4.0,   # b: a=-1
            np.zeros_like(bf), np.zeros_like(bf),
        ],
        axis=1,
    ).astype(np.float32)

    # blend matrices for the PE: out_p = sum_q M[p,q] * zc1_q over strip
    # slots (slot order [b|r|g]); M[p,q] = sf*d(p==q) + (1-sf)*w_{CHAN[q]}
    wvec = np.array(GRAY_W, np.float32)
    nimg_total = x16.shape[0]
    wdiag = np.zeros((nimg_total, 9, 128, 128), np.float16)
    idx = np.arange(128)
    for i in range(nimg_total):
        for p in range(3):
            for q in range(3):
                coeff = (sf[i] if p == q else 0.0) + (1.0 - sf[i]) * wvec[CHAN[q]]
                wdiag[i, p * 3 + q, idx, idx] = np.float16(coeff)

    nc = _get_nc()
    in_maps = [
        {"x": x16[k * NIMG:(k + 1) * NIMG], "fac": fac[k * NIMG:(k + 1) * NIMG],
         "wd": wdiag[k * NIMG:(k + 1) * NIMG]}
        for k in range(8)
    ]
    res = run_bass_kernel_spmd(nc, in_maps, core_ids=list(range(8)), trace=TRACE)
    if TRACE:
        _CACHE["last"] = res
    out = np.concatenate([res.results[k]["y"] for k in range(8)], axis=0)
    return out.astype(np.float32)
